# revision 2
# baseline (speedup 1.0000x reference)
"""CTC greedy decode (merge repeats, drop blank) on 8 Trainium2 cores.

Input : y_pred [256, 2048, 80] f32
Output: [256, 2048] int32, left-aligned decoded ids padded with -1.

Sharding: pure data-parallel, 32 sequences per core.

Per-core pipeline (B=32 seqs, N=65536 flat (b,t) rows), p-major layout:
chunk c holds rows [c*4096, (c+1)*4096); partition p owns the 32
consecutive rows p*32..p*32+31 (10KB contiguous per partition -> full-rate
DMA descriptors). Row n = c*4096 + p*32 + j  <->  seq 2c + p//64,
tau = (p%64)*32 + j: each (p, j-window) is 32 consecutive timesteps, so
no transpose is needed before the decode stage.

Argmax per row (exact, all on DVE fast paths):
  A: m = grouped max(y) via STT pairwise-max tree + small reduce (f32)
  B: d = y - m broadcast -> bf16 (exactly 0.0 iff y == m)
  C: v = d*2^38 + code,  code = 80-k (bf16; losers stay negative)
  D: r = grouped max(v) via bf16 STT tree + reduce = 80 - argmax
Ties pick the smaller k (larger code), matching the reference. m/r are
exported for a host-side safety net (repairs pathological rows exactly).

Decode stage (per quarter q = 4 chunks, S = r quarter tile [128,128]):
  keep = (r != 1) & (r != prev); prev across window edges via a
  shift-by-1-within-64 matmul. Composite keep*((31-j)*256 + 81 - r) is
  compacted per 32-column group by iterated Max8/match_replace; group
  offsets = seq base + cross-partition carry (strict-lower-triangular
  matmul within 64-blocks). One indirect DMA per group column
  accumulates the runs into the zero-initialized f32 outputs.
"""

import numpy as np

B, T, C = 256, 2048, 80
NCORES = 8
B_CORE = B // NCORES            # 32 seqs per core
N = B_CORE * T                  # 65536 flat rows per core
W = 32                          # tiles (rows per partition) per chunk
NCHUNK = 16
OUT_PAD = N + 40
SCALE = float(2.0 ** 38)

_cache = {}


def _build_nc():
    import concourse.bacc as bacc
    import concourse.mybir as mybir
    from concourse import bass
    from concourse.tile import TileContext

    f32 = mybir.dt.float32
    bf16 = mybir.dt.bfloat16
    i32 = mybir.dt.int32
    Alu = mybir.AluOpType
    Act = mybir.ActivationFunctionType

    nc = bacc.Bacc("TRN2")
    y = nc.dram_tensor("y", [N, C], f32, kind="ExternalInput")
    code2560 = nc.dram_tensor("code2560", [128, W * C], bf16,
                              kind="ExternalInput")
    p8 = nc.dram_tensor("p8", [128, 128], f32, kind="ExternalInput")
    sub64 = nc.dram_tensor("sub64", [128, 128], f32, kind="ExternalInput")
    t64 = nc.dram_tensor("t64", [128, 128], f32, kind="ExternalInput")
    seqb16 = nc.dram_tensor("seqb16", [128, 16], f32, kind="ExternalInput")
    outs = [nc.dram_tensor(f"out{i}", [1, OUT_PAD], f32,
                           kind="ExternalOutput") for i in range(4)]
    m_out = nc.dram_tensor("m_out", [128, NCHUNK * W], f32,
                           kind="ExternalOutput")
    r_out = nc.dram_tensor("r_out", [128, NCHUNK * W], f32,
                           kind="ExternalOutput")

    with TileContext(nc) as tc:
        with (
            tc.tile_pool(name="ypool", bufs=4) as ypool,
            tc.tile_pool(name="tree", bufs=2) as tpool,
            tc.tile_pool(name="dv", bufs=2) as dvpool,
            tc.tile_pool(name="persist", bufs=1) as ppool,
            tc.tile_pool(name="quarter", bufs=2) as qpool,
            tc.tile_pool(name="small", bufs=2) as smpool,
            tc.tile_pool(name="psum1", bufs=2, space="PSUM") as psum1,
        ):
            # constants
            code_sb = ppool.tile([128, W * C], bf16, tag="code")
            nc.scalar.dma_start(out=code_sb[:], in_=code2560[:])
            p8_sb = ppool.tile([128, 128], f32, tag="p8")
            nc.scalar.dma_start(out=p8_sb[:], in_=p8[:])
            sub64_sb = ppool.tile([128, 128], f32, tag="sub64")
            nc.scalar.dma_start(out=sub64_sb[:], in_=sub64[:])
            t64_sb = ppool.tile([128, 128], f32, tag="t64")
            nc.scalar.dma_start(out=t64_sb[:], in_=t64[:])
            seqb_sb = ppool.tile([128, 16], f32, tag="seqb16")
            nc.scalar.dma_start(out=seqb_sb[:], in_=seqb16[:])

            def chunk_pipe(c, mq, rq):
                """Stream chunk c, compute m and r columns into the
                quarter tiles mq/rq at columns [(c%4)*32, +32)."""
                g0 = (c % 4) * W
                yt = ypool.tile([128, W * C], f32, tag="y")
                src = bass.AP(y, c * 4096 * C, [[W * C, 128], [1, W * C]])
                nc.sync.dma_start(out=yt[:], in_=src)
                y3 = yt[:].rearrange("p (j k) -> p j k", k=C)

                # A: grouped max via pairwise STT-max tree (f32 2x mode)
                t1 = tpool.tile([128, W * 40], f32, tag="t1")
                t13 = t1[:].rearrange("p (j k) -> p j k", k=40)
                nc.vector.scalar_tensor_tensor(
                    out=t13, in0=y3[:, :, 0:40], scalar=0.0,
                    in1=y3[:, :, 40:80], op0=Alu.add, op1=Alu.max)
                t2 = tpool.tile([128, W * 20], f32, tag="t2")
                t23 = t2[:].rearrange("p (j k) -> p j k", k=20)
                nc.vector.scalar_tensor_tensor(
                    out=t23, in0=t13[:, :, 0:20], scalar=0.0,
                    in1=t13[:, :, 20:40], op0=Alu.add, op1=Alu.max)
                t3 = tpool.tile([128, W * 10], f32, tag="t3")
                t33 = t3[:].rearrange("p (j k) -> p j k", k=10)
                nc.vector.scalar_tensor_tensor(
                    out=t33, in0=t23[:, :, 0:10], scalar=0.0,
                    in1=t23[:, :, 10:20], op0=Alu.add, op1=Alu.max)
                t4 = tpool.tile([128, W * 5], f32, tag="t4")
                t43 = t4[:].rearrange("p (j k) -> p j k", k=5)
                nc.vector.scalar_tensor_tensor(
                    out=t43, in0=t33[:, :, 0:5], scalar=0.0,
                    in1=t33[:, :, 5:10], op0=Alu.add, op1=Alu.max)
                nc.vector.tensor_reduce(
                    out=mq[:, g0:g0 + W], in_=t43,
                    axis=mybir.AxisListType.X, op=Alu.max)

                # B: d = y - m (bf16 out; 0 exactly at the argmax)
                m3 = mq[:, g0:g0 + W].rearrange("p (j o) -> p j o", o=1) \
                    .to_broadcast([128, W, C])
                d = dvpool.tile([128, W * C], bf16, tag="d")
                d3 = d[:].rearrange("p (j k) -> p j k", k=C)
                nc.vector.scalar_tensor_tensor(
                    out=d3, in0=y3, scalar=0.0, in1=m3,
                    op0=Alu.add, op1=Alu.subtract)

                # C: v = d*2^38 + code (bf16 4x mode)
                v = dvpool.tile([128, W * C], bf16, tag="v")
                nc.vector.scalar_tensor_tensor(
                    out=v[:], in0=d[:], scalar=SCALE, in1=code_sb[:],
                    op0=Alu.mult, op1=Alu.add)

                # D: r = grouped max(v) via bf16 STT tree
                v3 = v[:].rearrange("p (j k) -> p j k", k=C)
                u1 = tpool.tile([128, W * 40], bf16, tag="u1")
                u13 = u1[:].rearrange("p (j k) -> p j k", k=40)
                nc.vector.scalar_tensor_tensor(
                    out=u13, in0=v3[:, :, 0:40], scalar=0.0,
                    in1=v3[:, :, 40:80], op0=Alu.add, op1=Alu.max)
                u2 = tpool.tile([128, W * 20], bf16, tag="u2")
                u23 = u2[:].rearrange("p (j k) -> p j k", k=20)
                nc.vector.scalar_tensor_tensor(
                    out=u23, in0=u13[:, :, 0:20], scalar=0.0,
                    in1=u13[:, :, 20:40], op0=Alu.add, op1=Alu.max)
                u3 = tpool.tile([128, W * 10], bf16, tag="u3")
                u33 = u3[:].rearrange("p (j k) -> p j k", k=10)
                nc.vector.scalar_tensor_tensor(
                    out=u33, in0=u23[:, :, 0:10], scalar=0.0,
                    in1=u23[:, :, 10:20], op0=Alu.add, op1=Alu.max)
                u4 = tpool.tile([128, W * 5], bf16, tag="u4")
                u43 = u4[:].rearrange("p (j k) -> p j k", k=5)
                nc.vector.scalar_tensor_tensor(
                    out=u43, in0=u33[:, :, 0:5], scalar=0.0,
                    in1=u33[:, :, 5:10], op0=Alu.add, op1=Alu.max)
                nc.vector.tensor_reduce(
                    out=rq[:, g0:g0 + W], in_=u43,
                    axis=mybir.AxisListType.X, op=Alu.max)

            def stage3(q, mq, rq):
                S = rq
                nc.sync.dma_start(
                    out=m_out[:, q * 128:(q + 1) * 128], in_=mq[:])
                nc.sync.dma_start(
                    out=r_out[:, q * 128:(q + 1) * 128], in_=S[:])

                # prev id at each 32-column window start: partition p-1's
                # col 31 of the same group (0 sentinel at seq starts)
                pc_ps = psum1.tile([128, 4], f32, space="PSUM", tag="pc")
                nc.tensor.matmul(
                    out=pc_ps[:], lhsT=sub64_sb[:],
                    rhs=S[:, 31:128:32], start=True, stop=True)
                pc = smpool.tile([128, 4], f32, tag="pcs")
                nc.scalar.activation(out=pc[:], in_=pc_ps[:], func=Act.Copy,
                                     bias=0.0, scale=1.0)

                # keep = (r != 1) & (r != prev)
                k1 = smpool.tile([128, 128], f32, tag="k1")
                nc.vector.tensor_scalar(
                    k1[:], S[:], 1.0, None, op0=Alu.not_equal)
                k2 = smpool.tile([128, 128], f32, tag="k2")
                nc.vector.tensor_tensor(
                    out=k2[:, 1:128], in0=S[:, 1:128], in1=S[:, 0:127],
                    op=Alu.not_equal)
                nc.vector.tensor_tensor(
                    out=k2[:, 0:128:32], in0=S[:, 0:128:32], in1=pc[:],
                    op=Alu.not_equal)
                keep = smpool.tile([128, 128], f32, tag="keep")
                nc.vector.tensor_tensor(
                    out=keep[:], in0=k1[:], in1=k2[:], op=Alu.mult)

                # composite = keep * ((31 - j)*256 + 81 - r)
                u1 = smpool.tile([128, 128], f32, tag="su1")
                nc.vector.scalar_tensor_tensor(
                    out=u1[:], in0=S[:], scalar=-1.0, in1=p8_sb[:],
                    op0=Alu.mult, op1=Alu.add)
                comp = smpool.tile([128, 128], f32, tag="comp")
                nc.vector.tensor_tensor(
                    out=comp[:], in0=u1[:], in1=keep[:], op=Alu.mult)

                # compact each 32-group: iterated Max8 + match_replace
                cruns = smpool.tile([128, 128], f32, tag="cruns")
                mrs = smpool.tile([128, 128], f32, tag="mrs")
                for g in range(4):
                    gs = slice(g * 32, (g + 1) * 32)
                    srcv = comp[:, gs]
                    for k in range(4):
                        ks = slice(g * 32 + k * 8, g * 32 + (k + 1) * 8)
                        nc.vector.max(out=cruns[:, ks], in_=srcv)
                        if k < 3:
                            nc.vector.match_replace(
                                out=mrs[:, gs], in_to_replace=cruns[:, ks],
                                in_values=srcv, imm_value=0.0)
                            srcv = mrs[:, gs]

                # group lengths -> cross-partition carry within 64-blocks
                ng = smpool.tile([128, 4], f32, tag="ng")
                nc.vector.tensor_reduce(
                    out=ng[:], in_=keep[:].rearrange("p (g e) -> p g e", e=32),
                    axis=mybir.AxisListType.X, op=Alu.add)
                ca_ps = psum1.tile([128, 4], f32, space="PSUM", tag="ca")
                nc.tensor.matmul(
                    out=ca_ps[:], lhsT=t64_sb[:], rhs=ng[:],
                    start=True, stop=True)
                orf = smpool.tile([128, 4], f32, tag="orf")
                nc.vector.scalar_tensor_tensor(
                    out=orf[:], in0=ca_ps[:], scalar=0.0,
                    in1=seqb_sb[:, q * 4:(q + 1) * 4],
                    op0=Alu.add, op1=Alu.add)
                off_i = smpool.tile([128, 4], i32, tag="off_i")
                nc.vector.tensor_copy(off_i[:], orf[:])

                for g in range(4):
                    nc.gpsimd.indirect_dma_start(
                        out=outs[g][:],
                        out_offset=bass.IndirectOffsetOnAxis(
                            ap=off_i[:, g:g + 1], axis=1),
                        in_=cruns[:, g * 32:(g + 1) * 32],
                        in_offset=None,
                        compute_op=Alu.add,
                    )

            prev = None
            for q in range(4):
                mq = qpool.tile([128, 128], f32, tag="mq")
                rq = qpool.tile([128, 128], f32, tag="rq")
                for g in range(4):
                    chunk_pipe(q * 4 + g, mq, rq)
                if prev is not None:
                    stage3(*prev)
                prev = (q, mq, rq)
            stage3(*prev)

    nc.finalize()
    return nc


def _consts():
    import ml_dtypes
    k = np.arange(128)
    code2560 = np.tile((C - np.arange(C)).astype(np.float64), W) \
        .astype(ml_dtypes.bfloat16)[None, :].repeat(128, axis=0)
    p8 = np.tile((31 - np.arange(128) % 32).astype(np.float32) * 256.0
                 + 81.0, (128, 1))
    sub64 = ((k[:, None] == (k[None, :] - 1)) &
             ((k[None, :] % 64) != 0)).astype(np.float32)
    t64 = ((k[:, None] // 64 == k[None, :] // 64) &
           (k[:, None] < k[None, :])).astype(np.float32)
    seqb16 = np.empty((128, 16), np.float32)
    for q in range(4):
        for g in range(4):
            seqb16[:, q * 4 + g] = (8 * q + 2 * g + (k >= 64)) * T
    return {"code2560": code2560, "p8": p8, "sub64": sub64, "t64": t64,
            "seqb16": seqb16}


def _reference_rows(y_rows):
    """Exact numpy replica of the reference decode for [n, T, C] rows."""
    n, t, c = y_rows.shape
    blank = c - 1
    ids = y_rows.argmax(axis=-1).astype(np.int32)
    prev = np.concatenate([np.full((n, 1), -1, np.int32), ids[:, :-1]], axis=1)
    keep = (ids != blank) & (ids != prev)
    pos = np.cumsum(keep, axis=1) - 1
    out = np.full((n, t), -1, np.int32)
    rows, cols = np.nonzero(keep)
    out[rows, pos[rows, cols]] = ids[rows, cols]
    return out


def kernel(y_pred: np.ndarray) -> np.ndarray:
    from concourse.bass_utils import run_bass_kernel_spmd

    if "nc" not in _cache:
        _cache["nc"] = _build_nc()
        _cache["consts"] = _consts()
    nc = _cache["nc"]
    consts = _cache["consts"]

    y_pred = np.ascontiguousarray(y_pred, dtype=np.float32)
    y_cores = y_pred.reshape(NCORES, N, C)
    in_maps = [dict(consts, y=y_cores[i]) for i in range(NCORES)]

    res = run_bass_kernel_spmd(nc, in_maps, core_ids=list(range(NCORES)))

    out_full = np.empty((B, T), np.int32)
    for i in range(NCORES):
        r = res.results[i]
        of = (r["out0"].ravel()[:N] + r["out1"].ravel()[:N] +
              r["out2"].ravel()[:N] + r["out3"].ravel()[:N])
        comp = np.rint(of).astype(np.int32)
        out_core = (comp % 256).reshape(B_CORE, T) - 1
        # --- host-side safety net for pathological rows ---
        # flat row n = c*4096 + p*32 + j  ->  m_out[p, c*32 + j]
        r_flat = r["r_out"].reshape(128, NCHUNK, W).transpose(1, 0, 2).ravel()
        m_flat = r["m_out"].reshape(128, NCHUNK, W).transpose(1, 0, 2).ravel()
        ids_dec = np.rint(C - r_flat).astype(np.int64)
        badrange = (ids_dec < 0) | (ids_dec > C - 1)
        idc = np.clip(ids_dec, 0, C - 1)
        y_flat = y_cores[i]
        bad = badrange | (y_flat[np.arange(N), idc] != m_flat)
        if bad.any():
            seqs = np.unique(np.nonzero(bad)[0] // T)
            fixed = _reference_rows(y_flat.reshape(B_CORE, T, C)[seqs])
            out_core[seqs] = fixed
        out_full[i * B_CORE:(i + 1) * B_CORE] = out_core
    return out_full


# revision 23
# speedup vs baseline: 1.5040x; 1.5040x over previous
"""CTC greedy decode (merge repeats, drop blank) on 8 Trainium2 cores.

Input : y_pred [256, 2048, 80] f32
Output: [256, 2048] int32, left-aligned decoded ids padded with -1.

Sharding: pure data-parallel, 32 sequences per core.

Per-core device pipeline (B=32 seqs, N=65536 flat (b,t) rows):
  1. Stream y in 16 chunks of [128, 32*80]; batched 3D reduce_max over the
     class axis -> m[128, 512] (per-row max).
  2. Per 128-row tile: scalar_tensor_tensor (y >= m) * w, w[c] = 80-c, with
     sum-accumulate -> r[128, 512] where r = 80 - argmax (exact when the row
     max is unique; tied rows are repaired on host via the m/r side outputs).
  3. PE-transpose r into S[t][block, tau] (time-major): partition n = block of
     128 consecutive tau, seq = (128*t + n) // 16. Compute keep flags; then
     compact each 8-element tau-group with the Max8 unit using a composite
     encoding keep * ((7 - tau%8)*256 + ids + 1): descending sort = stable
     compaction with zero tails. Group lengths -> prefix scan -> run offsets
     (PE matmul for the cross-partition block carry).
  4. One indirect-DMA per group column scatters 8-element runs (one run per
     partition) at their global offsets with accumulate-add onto the
     zero-initialized f32 output; zero tails make overlaps harmless. The host
     rounds, subtracts 1 (empty slots 0 -> -1).
"""

import numpy as np

B, T, C = 256, 2048, 80
NCORES = 8
B_CORE = B // NCORES            # 32 seqs per core
N = B_CORE * T                  # 65536 flat rows per core
TILES = N // 128                # 512
CHUNK_TILES = 32                # tiles per chunk
NCHUNK = TILES // CHUNK_TILES   # 16
OUT_PAD = N + 8

_cache = {}


def _build_nc():
    import concourse.bacc as bacc
    import concourse.mybir as mybir
    from concourse import bass
    from concourse.tile import TileContext

    f32 = mybir.dt.float32
    i32 = mybir.dt.int32
    Alu = mybir.AluOpType
    Act = mybir.ActivationFunctionType

    nc = bacc.Bacc("TRN2")
    y = nc.dram_tensor("y", [N, C], f32, kind="ExternalInput")
    bf16 = mybir.dt.bfloat16
    wcol = nc.dram_tensor("wcol", [C, 1], bf16, kind="ExternalInput")
    identb = nc.dram_tensor("identb", [128, 128], bf16, kind="ExternalInput")
    t16 = nc.dram_tensor("t16", [128, 128], f32, kind="ExternalInput")
    sub16 = nc.dram_tensor("sub16", [128, 128], f32, kind="ExternalInput")
    seqb = nc.dram_tensor("seqb", [128, 4], f32, kind="ExternalInput")
    ident = nc.dram_tensor("ident", [128, 128], f32, kind="ExternalInput")
    p8 = nc.dram_tensor("p8", [128, 128], f32, kind="ExternalInput")
    outs = [nc.dram_tensor(f"out{i}", [1, OUT_PAD], f32,
                            kind="ExternalOutput") for i in range(4)]
    m_out = nc.dram_tensor("m_out", [128, TILES], f32, kind="ExternalOutput")
    r_out = nc.dram_tensor("r_out", [128, TILES], f32, kind="ExternalOutput")

    # DRAM view: chunk c, partition p, tile-in-chunk j, class k
    y_re = y[:].rearrange("(c j p) k -> c p j k", c=NCHUNK, j=CHUNK_TILES, p=128)

    with TileContext(nc) as tc:
        with (
            tc.tile_pool(name="ypool", bufs=6) as ypool,
            tc.tile_pool(name="scratch", bufs=6) as spool,
            tc.tile_pool(name="persist", bufs=1) as ppool,
            tc.tile_pool(name="small", bufs=3) as smpool,
            tc.tile_pool(name="psum", bufs=3, space="PSUM") as psum,
            tc.tile_pool(name="psum1", bufs=1, space="PSUM") as psum1,
            tc.tile_pool(name="psumr", bufs=2, space="PSUM") as psumr,
        ):
            # constants
            w_sb = ppool.tile([C, 1], bf16, tag="w")
            nc.scalar.dma_start(out=w_sb[:], in_=wcol[:])
            idb_sb = ppool.tile([128, 128], bf16, tag="identb")
            nc.scalar.dma_start(out=idb_sb[:], in_=identb[:])
            t16_sb = ppool.tile([128, 128], f32, tag="t16")
            nc.scalar.dma_start(out=t16_sb[:], in_=t16[:])
            sub16_sb = ppool.tile([128, 128], f32, tag="sub16")
            nc.scalar.dma_start(out=sub16_sb[:], in_=sub16[:])
            seqb_sb = ppool.tile([128, 4], f32, tag="seqb")
            nc.scalar.dma_start(out=seqb_sb[:], in_=seqb[:])
            id_sb = ppool.tile([128, 128], f32, tag="ident")
            nc.scalar.dma_start(out=id_sb[:], in_=ident[:])
            p8_sb = ppool.tile([128, 128], f32, tag="p8")
            nc.scalar.dma_start(out=p8_sb[:], in_=p8[:])

            # chunk size schedule: small chunks at the very start (fast
            # pipeline fill) and at the very end (short drain chain)
            QCHUNKS = {0: [8, 8, 16, 32, 32, 32],
                       1: [32, 32, 32, 32],
                       2: [32, 32, 32, 32],
                       3: [32, 32, 32, 16, 8, 8]}
            # ---- interleaved: stream chunks; after each quarter of r is
            # complete, run that quarter's decode+compact+scatter ----
            def stream_quarter(t):
                r_ps = psumr.tile([128, 128], f32, space="PSUM", tag="rq_ps")
                tile0 = t * 128
                jq0 = 0
                for ct in QCHUNKS[t]:
                    yt = ypool.tile([128, CHUNK_TILES * C], f32, tag="y")
                    src = bass.AP(
                        y, (tile0 + jq0) * 128 * C,
                        [[C, 128], [128 * C, ct], [1, C]])
                    nc.sync.dma_start(out=yt[:, :ct * C], in_=src)
                    y3 = yt[:, :ct * C].rearrange("p (j k) -> p j k", k=C)
                    c0 = tile0 + jq0
                    m_t = spool.tile([128, CHUNK_TILES], f32, tag="m")
                    nc.vector.tensor_reduce(
                        out=m_t[:, :ct], in_=y3,
                        axis=mybir.AxisListType.X, op=Alu.max,
                    )
                    nc.sync.dma_start(
                        out=m_out[:, c0:c0 + ct], in_=m_t[:, :ct])
                    # candidate mask (bf16, exact 0/1), batched over the chunk
                    eq = spool.tile([128, CHUNK_TILES * C], bf16, tag="eq")
                    m3 = m_t[:, :ct].rearrange("p (j o) -> p j o", o=1) \
                        .to_broadcast([128, ct, C])
                    nc.vector.tensor_tensor(
                        out=eq[:, :ct * C].rearrange("p (j k) -> p j k", k=C),
                        in0=y3, in1=m3, op=Alu.is_ge)
                    # r[tile] = sum_c eq[:, c] * w[c] on the TensorEngine:
                    # transpose eq per tile, then a 1-column matvec into the
                    # quarter's PSUM accumulator
                    eqv = eq[:, :ct * C].rearrange("p (j k) -> p j k", k=C)
                    for grp in range(ct // 8):
                        tr_ps = psum.tile([80, 1024], bf16, space="PSUM",
                                          tag="tr")
                        for j8 in range(8):
                            j = grp * 8 + j8
                            nc.tensor.transpose(
                                out=tr_ps[:, j8 * 128:(j8 + 1) * 128],
                                in_=eqv[:, j, :], identity=idb_sb[:])
                        eqT = spool.tile([80, 1024], bf16, tag="eqT")
                        nc.scalar.activation(
                            out=eqT[:], in_=tr_ps[:], func=Act.Copy,
                            bias=0.0, scale=1.0)
                        for j8 in range(8):
                            jq = jq0 + grp * 8 + j8
                            nc.tensor.matmul(
                                out=r_ps[:, jq:jq + 1],
                                lhsT=eqT[:, j8 * 128:(j8 + 1) * 128],
                                rhs=w_sb[:], start=True, stop=True)
                    jq0 += ct
                r_q = spool.tile([128, 128], f32, tag="rq")
                nc.scalar.activation(
                    out=r_q[:], in_=r_ps[:], func=Act.Copy, bias=0.0,
                    scale=1.0)
                nc.sync.dma_start(
                    out=r_out[:, t * 128:(t + 1) * 128], in_=r_q[:])

                return r_q

            def stage3a(t, r_q):
                rT_ps = psum1.tile([128, 128], f32, space="PSUM", tag="rT")
                nc.tensor.transpose(
                    out=rT_ps[:], in_=r_q[:],
                    identity=id_sb[:],
                )
                S = smpool.tile([128, 128], f32, tag="S")
                nc.scalar.activation(
                    out=S[:], in_=rT_ps[:], func=Act.Copy, bias=0.0, scale=1.0)

                # prevcol[n] = S[n-1, 127] if n%16 else 0 (seq-start sentinel)
                pc_ps = psum1.tile([128, 1], f32, space="PSUM", tag="pc")
                nc.tensor.matmul(
                    out=pc_ps[:], lhsT=sub16_sb[:], rhs=S[:, 127:128],
                    start=True, stop=True,
                )
                pc = smpool.tile([128, 1], f32, tag="pcs")
                nc.scalar.activation(out=pc[:], in_=pc_ps[:], func=Act.Copy,
                                     bias=0.0, scale=1.0)

                return S, pc

            def stage3b(t, S, pc):
                # keep = (r != 1) & (r != prev)
                k1 = smpool.tile([128, 128], f32, tag="k1")
                nc.vector.tensor_scalar(
                    k1[:], S[:], 1.0, None, op0=Alu.not_equal)
                k2 = smpool.tile([128, 128], f32, tag="k2")
                nc.vector.tensor_tensor(
                    out=k2[:, 1:128], in0=S[:, 1:128], in1=S[:, 0:127],
                    op=Alu.not_equal)
                nc.vector.tensor_tensor(
                    out=k2[:, 0:1], in0=S[:, 0:1], in1=pc[:],
                    op=Alu.not_equal)
                keep = smpool.tile([128, 128], f32, tag="keep")
                nc.vector.tensor_tensor(
                    out=keep[:], in0=k1[:], in1=k2[:], op=Alu.mult)

                # composite = keep * ((31 - tau%32)*256 + ids + 1)
                # p8 const already includes the +81 (= ids+1 = 81 - r)
                u1 = smpool.tile([128, 128], f32, tag="u1")
                nc.vector.scalar_tensor_tensor(
                    out=u1[:], in0=S[:], scalar=-1.0, in1=p8_sb[:],
                    op0=Alu.mult, op1=Alu.add)
                comp = smpool.tile([128, 128], f32, tag="comp")
                nc.vector.tensor_tensor(
                    out=comp[:], in0=u1[:], in1=keep[:], op=Alu.mult)

                # compact each 32-group: iterated Max8 + match_replace
                # (descending sort with zero tails)
                cruns = smpool.tile([128, 128], f32, tag="cruns")
                mrs = smpool.tile([128, 128], f32, tag="mrs")
                for g in range(4):
                    gs = slice(g * 32, (g + 1) * 32)
                    src = comp[:, gs]
                    for k in range(4):
                        ks = slice(g * 32 + k * 8, g * 32 + (k + 1) * 8)
                        nc.vector.max(out=cruns[:, ks], in_=src)
                        if k < 3:
                            nc.vector.match_replace(
                                out=mrs[:, gs], in_to_replace=cruns[:, ks],
                                in_values=src, imm_value=0.0)
                            src = mrs[:, gs]

                # group lengths and exclusive scan -> within-partition offsets
                ng = smpool.tile([128, 4], f32, tag="ng")
                nc.vector.tensor_reduce(
                    out=ng[:], in_=keep[:].rearrange("p (g e) -> p g e", e=32),
                    axis=mybir.AxisListType.X, op=Alu.add)
                og = smpool.tile([128, 5], f32, tag="og")
                nc.vector.memset(og[:, 0:1], 0.0)
                nc.vector.tensor_tensor_scan(
                    out=og[:, 1:5], data0=ng[:], data1=ng[:], initial=0.0,
                    op0=Alu.add, op1=Alu.bypass)

                # cross-partition carry within each 16-partition seq group
                ca_ps = psum1.tile([128, 1], f32, space="PSUM", tag="ca")
                nc.tensor.matmul(
                    out=ca_ps[:], lhsT=t16_sb[:], rhs=og[:, 4:5],
                    start=True, stop=True,
                )
                # c3 = carry + seqbase
                c3 = smpool.tile([128, 1], f32, tag="c3")
                nc.vector.scalar_tensor_tensor(
                    out=c3[:], in0=ca_ps[:], scalar=0.0,
                    in1=seqb_sb[:, t:t + 1], op0=Alu.add, op1=Alu.add)

                # run offsets
                orf = smpool.tile([128, 4], f32, tag="orf")
                nc.vector.tensor_scalar(
                    orf[:], og[:, 0:4], c3[:], None, op0=Alu.add)
                off_i = smpool.tile([128, 4], i32, tag="off_i")
                nc.vector.tensor_copy(off_i[:], orf[:])

                for g in range(4):
                    nc.gpsimd.indirect_dma_start(
                        out=outs[g][:],
                        out_offset=bass.IndirectOffsetOnAxis(
                            ap=off_i[:, g:g + 1], axis=1),
                        in_=cruns[:, g * 32:(g + 1) * 32],
                        in_offset=None,
                        compute_op=Alu.add,
                    )


            prev = None
            for t in range(4):
                rq = stream_quarter(t)
                sp = stage3a(t, rq)
                if prev is not None:
                    stage3b(prev[0], *prev[1])
                prev = (t, sp)
            stage3b(prev[0], *prev[1])

    nc.finalize()
    return nc


def _consts():
    import ml_dtypes
    k = np.arange(128)
    wcol = (C - np.arange(C)).astype(ml_dtypes.bfloat16).reshape(C, 1)
    identb = np.eye(128, dtype=ml_dtypes.bfloat16)
    t16 = (((k[:, None] // 16) == (k[None, :] // 16)) &
           (k[:, None] < k[None, :])).astype(np.float32)
    sub16 = ((k[:, None] == (k[None, :] - 1)) &
             ((k[None, :] % 16) != 0)).astype(np.float32)
    seqb = np.empty((128, 4), np.float32)
    for t in range(4):
        seqb[:, t] = ((128 * t + k) // 16) * T
    ident = np.eye(128, dtype=np.float32)
    p8 = np.tile((31 - np.arange(128) % 32).astype(np.float32) * 256.0
                 + 81.0, (128, 1))
    return {"wcol": wcol, "identb": identb, "t16": t16, "sub16": sub16,
            "seqb": seqb, "ident": ident, "p8": p8}


def _reference_rows(y_rows):
    """Exact numpy replica of the reference decode for [n, T, C] rows."""
    n, t, c = y_rows.shape
    blank = c - 1
    ids = y_rows.argmax(axis=-1).astype(np.int32)
    prev = np.concatenate([np.full((n, 1), -1, np.int32), ids[:, :-1]], axis=1)
    keep = (ids != blank) & (ids != prev)
    pos = np.cumsum(keep, axis=1) - 1
    out = np.full((n, t), -1, np.int32)
    rows, cols = np.nonzero(keep)
    out[rows, pos[rows, cols]] = ids[rows, cols]
    return out


def kernel(y_pred: np.ndarray) -> np.ndarray:
    from concourse.bass_utils import run_bass_kernel_spmd

    if "nc" not in _cache:
        _cache["nc"] = _build_nc()
        _cache["consts"] = _consts()
    nc = _cache["nc"]
    consts = _cache["consts"]

    y_pred = np.ascontiguousarray(y_pred, dtype=np.float32)
    y_cores = y_pred.reshape(NCORES, N, C)
    in_maps = [dict(consts, y=y_cores[i]) for i in range(NCORES)]

    res = run_bass_kernel_spmd(nc, in_maps, core_ids=list(range(NCORES)))

    out_full = np.empty((B, T), np.int32)
    for i in range(NCORES):
        r = res.results[i]
        of = (r["out0"].ravel()[:N] + r["out1"].ravel()[:N] +
              r["out2"].ravel()[:N] + r["out3"].ravel()[:N])
        comp = np.rint(of).astype(np.int32)
        out_core = (comp % 256).reshape(B_CORE, T) - 1
        # --- host-side verification/repair for tied-max rows ---
        # flat row g lives at (g % 128, g // 128) in the [128, TILES] outputs
        r_flat = np.ascontiguousarray(r["r_out"].T).ravel()
        m_flat = np.ascontiguousarray(r["m_out"].T).ravel()
        ids_dec = np.rint(C - r_flat).astype(np.int64)
        badrange = (ids_dec < 0) | (ids_dec > C - 1)
        idc = np.clip(ids_dec, 0, C - 1)
        y_flat = y_cores[i]
        bad = badrange | (y_flat[np.arange(N), idc] != m_flat)
        if bad.any():
            seqs = np.unique(np.nonzero(bad)[0] // T)
            fixed = _reference_rows(y_flat.reshape(B_CORE, T, C)[seqs])
            out_core[seqs] = fixed
        out_full[i * B_CORE:(i + 1) * B_CORE] = out_core
    return out_full



# revision 26
# speedup vs baseline: 1.5052x; 1.0008x over previous
"""CTC greedy decode (merge repeats, drop blank) on 8 Trainium2 cores.

Input : y_pred [256, 2048, 80] f32
Output: [256, 2048] int32, left-aligned decoded ids padded with -1.

Sharding: pure data-parallel, 32 sequences per core.

Per-core device pipeline (B=32 seqs, N=65536 flat (b,t) rows):
  1. Stream y in 16 chunks of [128, 32*80]; batched 3D reduce_max over the
     class axis -> m[128, 512] (per-row max).
  2. Per 128-row tile: scalar_tensor_tensor (y >= m) * w, w[c] = 80-c, with
     sum-accumulate -> r[128, 512] where r = 80 - argmax (exact when the row
     max is unique; tied rows are repaired on host via the m/r side outputs).
  3. PE-transpose r into S[t][block, tau] (time-major): partition n = block of
     128 consecutive tau, seq = (128*t + n) // 16. Compute keep flags; then
     compact each 8-element tau-group with the Max8 unit using a composite
     encoding keep * ((7 - tau%8)*256 + ids + 1): descending sort = stable
     compaction with zero tails. Group lengths -> prefix scan -> run offsets
     (PE matmul for the cross-partition block carry).
  4. One indirect-DMA per group column scatters 8-element runs (one run per
     partition) at their global offsets with accumulate-add onto the
     zero-initialized f32 output; zero tails make overlaps harmless. The host
     rounds, subtracts 1 (empty slots 0 -> -1).
"""

import numpy as np

B, T, C = 256, 2048, 80
NCORES = 8
B_CORE = B // NCORES            # 32 seqs per core
N = B_CORE * T                  # 65536 flat rows per core
TILES = N // 128                # 512
CHUNK_TILES = 32                # tiles per chunk
NCHUNK = TILES // CHUNK_TILES   # 16
OUT_PAD = N + 8

_cache = {}


def _build_nc():
    import concourse.bacc as bacc
    import concourse.mybir as mybir
    from concourse import bass
    from concourse.tile import TileContext

    f32 = mybir.dt.float32
    i32 = mybir.dt.int32
    Alu = mybir.AluOpType
    Act = mybir.ActivationFunctionType

    nc = bacc.Bacc("TRN2")
    y = nc.dram_tensor("y", [N, C], f32, kind="ExternalInput")
    bf16 = mybir.dt.bfloat16
    wcol = nc.dram_tensor("wcol", [C, 1], bf16, kind="ExternalInput")
    identb = nc.dram_tensor("identb", [128, 128], bf16, kind="ExternalInput")
    t16 = nc.dram_tensor("t16", [128, 128], f32, kind="ExternalInput")
    sub16 = nc.dram_tensor("sub16", [128, 128], f32, kind="ExternalInput")
    seqb = nc.dram_tensor("seqb", [128, 4], f32, kind="ExternalInput")
    ident = nc.dram_tensor("ident", [128, 128], f32, kind="ExternalInput")
    p8 = nc.dram_tensor("p8", [128, 128], f32, kind="ExternalInput")
    outs = [nc.dram_tensor(f"out{i}", [1, OUT_PAD], f32,
                            kind="ExternalOutput") for i in range(4)]
    m_out = nc.dram_tensor("m_out", [128, TILES], f32, kind="ExternalOutput")
    r_out = nc.dram_tensor("r_out", [128, TILES], f32, kind="ExternalOutput")

    # DRAM view: chunk c, partition p, tile-in-chunk j, class k
    y_re = y[:].rearrange("(c j p) k -> c p j k", c=NCHUNK, j=CHUNK_TILES, p=128)

    with TileContext(nc) as tc:
        with (
            tc.tile_pool(name="ypool", bufs=6) as ypool,
            tc.tile_pool(name="scratch", bufs=6) as spool,
            tc.tile_pool(name="persist", bufs=1) as ppool,
            tc.tile_pool(name="quarter", bufs=2) as qpool,
            tc.tile_pool(name="small", bufs=3) as smpool,
            tc.tile_pool(name="psum", bufs=3, space="PSUM") as psum,
            tc.tile_pool(name="psum1", bufs=1, space="PSUM") as psum1,
            tc.tile_pool(name="psumr", bufs=2, space="PSUM") as psumr,
        ):
            # constants
            w_sb = ppool.tile([C, 1], bf16, tag="w")
            nc.scalar.dma_start(out=w_sb[:], in_=wcol[:])
            idb_sb = ppool.tile([128, 128], bf16, tag="identb")
            nc.scalar.dma_start(out=idb_sb[:], in_=identb[:])
            t16_sb = ppool.tile([128, 128], f32, tag="t16")
            nc.scalar.dma_start(out=t16_sb[:], in_=t16[:])
            sub16_sb = ppool.tile([128, 128], f32, tag="sub16")
            nc.scalar.dma_start(out=sub16_sb[:], in_=sub16[:])
            seqb_sb = ppool.tile([128, 4], f32, tag="seqb")
            nc.scalar.dma_start(out=seqb_sb[:], in_=seqb[:])
            id_sb = ppool.tile([128, 128], f32, tag="ident")
            nc.scalar.dma_start(out=id_sb[:], in_=ident[:])
            p8_sb = ppool.tile([128, 128], f32, tag="p8")
            nc.scalar.dma_start(out=p8_sb[:], in_=p8[:])

            # chunk size schedule: small chunks at the very start (fast
            # pipeline fill) and at the very end (short drain chain)
            QCHUNKS = {0: [8, 8, 16, 32, 32, 32],
                       1: [32, 32, 32, 32],
                       2: [32, 32, 32, 32],
                       3: [32, 32, 32, 16, 8, 8]}
            # ---- interleaved: stream chunks; after each quarter of r is
            # complete, run that quarter's decode+compact+scatter ----
            def stream_quarter(t):
                # m accumulates in a per-quarter tile; its m_out store is
                # issued once per quarter from the Act sequencer so the
                # sync queue stays a pure y-DMA prefetch stream (a store
                # there head-of-line blocks the next chunk's DMA behind
                # this chunk's reduce).
                r_ps = psumr.tile([128, 128], f32, space="PSUM", tag="rq_ps")
                mq = qpool.tile([128, 128], f32, tag="mq")
                tile0 = t * 128
                jq0 = 0
                for ct in QCHUNKS[t]:
                    yt = ypool.tile([128, CHUNK_TILES * C], f32, tag="y")
                    src = bass.AP(
                        y, (tile0 + jq0) * 128 * C,
                        [[C, 128], [128 * C, ct], [1, C]])
                    nc.sync.dma_start(out=yt[:, :ct * C], in_=src)
                    y3 = yt[:, :ct * C].rearrange("p (j k) -> p j k", k=C)
                    nc.vector.tensor_reduce(
                        out=mq[:, jq0:jq0 + ct], in_=y3,
                        axis=mybir.AxisListType.X, op=Alu.max,
                    )
                    # candidate mask (bf16, exact 0/1), batched over the chunk
                    eq = spool.tile([128, CHUNK_TILES * C], bf16, tag="eq")
                    m3 = mq[:, jq0:jq0 + ct].rearrange("p (j o) -> p j o", o=1) \
                        .to_broadcast([128, ct, C])
                    nc.vector.tensor_tensor(
                        out=eq[:, :ct * C].rearrange("p (j k) -> p j k", k=C),
                        in0=y3, in1=m3, op=Alu.is_ge)
                    # r[tile] = sum_c eq[:, c] * w[c] on the TensorEngine:
                    # transpose eq per tile, then a 1-column matvec into the
                    # quarter's PSUM accumulator
                    eqv = eq[:, :ct * C].rearrange("p (j k) -> p j k", k=C)
                    for grp in range(ct // 8):
                        tr_ps = psum.tile([80, 1024], bf16, space="PSUM",
                                          tag="tr")
                        for j8 in range(8):
                            j = grp * 8 + j8
                            nc.tensor.transpose(
                                out=tr_ps[:, j8 * 128:(j8 + 1) * 128],
                                in_=eqv[:, j, :], identity=idb_sb[:])
                        eqT = spool.tile([80, 1024], bf16, tag="eqT")
                        nc.scalar.activation(
                            out=eqT[:], in_=tr_ps[:], func=Act.Copy,
                            bias=0.0, scale=1.0)
                        for j8 in range(8):
                            jq = jq0 + grp * 8 + j8
                            nc.tensor.matmul(
                                out=r_ps[:, jq:jq + 1],
                                lhsT=eqT[:, j8 * 128:(j8 + 1) * 128],
                                rhs=w_sb[:], start=True, stop=True)
                    jq0 += ct
                nc.scalar.dma_start(
                    out=m_out[:, t * 128:(t + 1) * 128], in_=mq[:])
                r_q = spool.tile([128, 128], f32, tag="rq")
                nc.scalar.activation(
                    out=r_q[:], in_=r_ps[:], func=Act.Copy, bias=0.0,
                    scale=1.0)
                nc.scalar.dma_start(
                    out=r_out[:, t * 128:(t + 1) * 128], in_=r_q[:])

                return r_q

            def stage3a(t, r_q):
                rT_ps = psum1.tile([128, 128], f32, space="PSUM", tag="rT")
                nc.tensor.transpose(
                    out=rT_ps[:], in_=r_q[:],
                    identity=id_sb[:],
                )
                S = smpool.tile([128, 128], f32, tag="S")
                nc.scalar.activation(
                    out=S[:], in_=rT_ps[:], func=Act.Copy, bias=0.0, scale=1.0)

                # prevcol[n] = S[n-1, 127] if n%16 else 0 (seq-start sentinel)
                pc_ps = psum1.tile([128, 1], f32, space="PSUM", tag="pc")
                nc.tensor.matmul(
                    out=pc_ps[:], lhsT=sub16_sb[:], rhs=S[:, 127:128],
                    start=True, stop=True,
                )
                pc = smpool.tile([128, 1], f32, tag="pcs")
                nc.scalar.activation(out=pc[:], in_=pc_ps[:], func=Act.Copy,
                                     bias=0.0, scale=1.0)

                return S, pc

            def stage3b(t, S, pc):
                # keep = (r != 1) & (r != prev)
                k1 = smpool.tile([128, 128], f32, tag="k1")
                nc.vector.tensor_scalar(
                    k1[:], S[:], 1.0, None, op0=Alu.not_equal)
                k2 = smpool.tile([128, 128], f32, tag="k2")
                nc.vector.tensor_tensor(
                    out=k2[:, 1:128], in0=S[:, 1:128], in1=S[:, 0:127],
                    op=Alu.not_equal)
                nc.vector.tensor_tensor(
                    out=k2[:, 0:1], in0=S[:, 0:1], in1=pc[:],
                    op=Alu.not_equal)
                keep = smpool.tile([128, 128], f32, tag="keep")
                nc.vector.tensor_tensor(
                    out=keep[:], in0=k1[:], in1=k2[:], op=Alu.mult)

                # composite = keep * ((31 - tau%32)*256 + ids + 1)
                # p8 const already includes the +81 (= ids+1 = 81 - r)
                u1 = smpool.tile([128, 128], f32, tag="u1")
                nc.vector.scalar_tensor_tensor(
                    out=u1[:], in0=S[:], scalar=-1.0, in1=p8_sb[:],
                    op0=Alu.mult, op1=Alu.add)
                comp = smpool.tile([128, 128], f32, tag="comp")
                nc.vector.tensor_tensor(
                    out=comp[:], in0=u1[:], in1=keep[:], op=Alu.mult)

                # compact each 32-group: iterated Max8 + match_replace
                # (descending sort with zero tails)
                cruns = smpool.tile([128, 128], f32, tag="cruns")
                mrs = smpool.tile([128, 128], f32, tag="mrs")
                for g in range(4):
                    gs = slice(g * 32, (g + 1) * 32)
                    src = comp[:, gs]
                    for k in range(4):
                        ks = slice(g * 32 + k * 8, g * 32 + (k + 1) * 8)
                        nc.vector.max(out=cruns[:, ks], in_=src)
                        if k < 3:
                            nc.vector.match_replace(
                                out=mrs[:, gs], in_to_replace=cruns[:, ks],
                                in_values=src, imm_value=0.0)
                            src = mrs[:, gs]

                # group lengths and exclusive scan -> within-partition offsets
                ng = smpool.tile([128, 4], f32, tag="ng")
                nc.vector.tensor_reduce(
                    out=ng[:], in_=keep[:].rearrange("p (g e) -> p g e", e=32),
                    axis=mybir.AxisListType.X, op=Alu.add)
                og = smpool.tile([128, 5], f32, tag="og")
                nc.vector.memset(og[:, 0:1], 0.0)
                nc.vector.tensor_tensor_scan(
                    out=og[:, 1:5], data0=ng[:], data1=ng[:], initial=0.0,
                    op0=Alu.add, op1=Alu.bypass)

                # cross-partition carry within each 16-partition seq group
                ca_ps = psum1.tile([128, 1], f32, space="PSUM", tag="ca")
                nc.tensor.matmul(
                    out=ca_ps[:], lhsT=t16_sb[:], rhs=og[:, 4:5],
                    start=True, stop=True,
                )
                # c3 = carry + seqbase
                c3 = smpool.tile([128, 1], f32, tag="c3")
                nc.vector.scalar_tensor_tensor(
                    out=c3[:], in0=ca_ps[:], scalar=0.0,
                    in1=seqb_sb[:, t:t + 1], op0=Alu.add, op1=Alu.add)

                # run offsets
                orf = smpool.tile([128, 4], f32, tag="orf")
                nc.vector.tensor_scalar(
                    orf[:], og[:, 0:4], c3[:], None, op0=Alu.add)
                off_i = smpool.tile([128, 4], i32, tag="off_i")
                nc.vector.tensor_copy(off_i[:], orf[:])

                for g in range(4):
                    nc.gpsimd.indirect_dma_start(
                        out=outs[g][:],
                        out_offset=bass.IndirectOffsetOnAxis(
                            ap=off_i[:, g:g + 1], axis=1),
                        in_=cruns[:, g * 32:(g + 1) * 32],
                        in_offset=None,
                        compute_op=Alu.add,
                    )


            prev = None
            for t in range(4):
                rq = stream_quarter(t)
                sp = stage3a(t, rq)
                if prev is not None:
                    stage3b(prev[0], *prev[1])
                prev = (t, sp)
            stage3b(prev[0], *prev[1])

    nc.finalize()
    return nc


def _consts():
    import ml_dtypes
    k = np.arange(128)
    wcol = (C - np.arange(C)).astype(ml_dtypes.bfloat16).reshape(C, 1)
    identb = np.eye(128, dtype=ml_dtypes.bfloat16)
    t16 = (((k[:, None] // 16) == (k[None, :] // 16)) &
           (k[:, None] < k[None, :])).astype(np.float32)
    sub16 = ((k[:, None] == (k[None, :] - 1)) &
             ((k[None, :] % 16) != 0)).astype(np.float32)
    seqb = np.empty((128, 4), np.float32)
    for t in range(4):
        seqb[:, t] = ((128 * t + k) // 16) * T
    ident = np.eye(128, dtype=np.float32)
    p8 = np.tile((31 - np.arange(128) % 32).astype(np.float32) * 256.0
                 + 81.0, (128, 1))
    return {"wcol": wcol, "identb": identb, "t16": t16, "sub16": sub16,
            "seqb": seqb, "ident": ident, "p8": p8}


def _reference_rows(y_rows):
    """Exact numpy replica of the reference decode for [n, T, C] rows."""
    n, t, c = y_rows.shape
    blank = c - 1
    ids = y_rows.argmax(axis=-1).astype(np.int32)
    prev = np.concatenate([np.full((n, 1), -1, np.int32), ids[:, :-1]], axis=1)
    keep = (ids != blank) & (ids != prev)
    pos = np.cumsum(keep, axis=1) - 1
    out = np.full((n, t), -1, np.int32)
    rows, cols = np.nonzero(keep)
    out[rows, pos[rows, cols]] = ids[rows, cols]
    return out


def kernel(y_pred: np.ndarray) -> np.ndarray:
    from concourse.bass_utils import run_bass_kernel_spmd

    if "nc" not in _cache:
        _cache["nc"] = _build_nc()
        _cache["consts"] = _consts()
    nc = _cache["nc"]
    consts = _cache["consts"]

    y_pred = np.ascontiguousarray(y_pred, dtype=np.float32)
    y_cores = y_pred.reshape(NCORES, N, C)
    in_maps = [dict(consts, y=y_cores[i]) for i in range(NCORES)]

    res = run_bass_kernel_spmd(nc, in_maps, core_ids=list(range(NCORES)))

    out_full = np.empty((B, T), np.int32)
    for i in range(NCORES):
        r = res.results[i]
        of = (r["out0"].ravel()[:N] + r["out1"].ravel()[:N] +
              r["out2"].ravel()[:N] + r["out3"].ravel()[:N])
        comp = np.rint(of).astype(np.int32)
        out_core = (comp % 256).reshape(B_CORE, T) - 1
        # --- host-side verification/repair for tied-max rows ---
        # flat row g lives at (g % 128, g // 128) in the [128, TILES] outputs
        r_flat = np.ascontiguousarray(r["r_out"].T).ravel()
        m_flat = np.ascontiguousarray(r["m_out"].T).ravel()
        ids_dec = np.rint(C - r_flat).astype(np.int64)
        badrange = (ids_dec < 0) | (ids_dec > C - 1)
        idc = np.clip(ids_dec, 0, C - 1)
        y_flat = y_cores[i]
        bad = badrange | (y_flat[np.arange(N), idc] != m_flat)
        if bad.any():
            seqs = np.unique(np.nonzero(bad)[0] // T)
            fixed = _reference_rows(y_flat.reshape(B_CORE, T, C)[seqs])
            out_core[seqs] = fixed
        out_full[i * B_CORE:(i + 1) * B_CORE] = out_core
    return out_full

